# revision 4
# baseline (speedup 1.0000x reference)
"""Causal self-attention (B=4, T=2048, D=1024, H=16) on 8 TRN2 NeuronCores.

Sharding: core c handles batch b=c//2 and head-group g=c%2 (8 heads).
Each core computes its heads' attention + a partial output projection
(contraction over its 512 attn channels); the host sums the two partials
per batch and adds b_out.

v3 (from v2 @362us): trace-driven fixes
 - sAB psum ring is now dedicated (rope t2 no longer steals its slots):
   t2 is computed IN-PLACE into the qk-proj psum tile, so S(kb+1) truly
   double-buffers against exp(kb).
 - x chunks are DMA-prefetched one chunk ahead; initial const DMAs go on
   the scalar-engine HWDGE queue in parallel with wqk/x on sync.
 - rope tables are bf16 (halves their DMA); head-B rope combines run on
   GpSimd to cut the DVE critical path.
 - norm chain slimmed: reciprocal reads the Z row straight from PSUM,
   one zb tile with two half broadcasts, no intermediate copies.
 - pair-lookahead gating: next pair's first chunks are pumped during the
   current pair's last q-chunk, so S never waits on rope at boundaries.

Per-core device pipeline (per head-pair p, heads packed 2/128-partitions):
  qk-proj   qkT[ch,T] = wqk.T @ xT  (bf16, ch-major), rope via DVE/Pool
  v-proj    V[t, 2*65] = xT.T @ wv_aug  (65th col per head = ones -> Z)
  S^T       [k,q] = k'^T q' per head, 2 heads in PE quadrants (K=64)
  softmax   exp((S)*0.125) in ONE ACT instr per block (both heads),
            bf16 out; normalizer Z from the V ones col
  PV        attn_aug^T[65,q] = V_aug^T @ E^T accumulated over k blocks
  norm      rz = 1/Z (DVE recip from psum), partition-bcast (Pool),
            attnT = pv * rz -> bf16
  out-proj  out[q,o] = attnT.T @ wo  (partial; host adds pair partials)
"""
import sys
from collections import deque
import numpy as np

for _p in ("/opt/trn_rl_repo", "/root/.axon_site/_ro/trn_rl_repo"):
    if _p not in sys.path:
        sys.path.append(_p)

import ml_dtypes
import concourse.bass as bass
import concourse.bacc as bacc
import concourse.tile as tile
import concourse.mybir as mybir
from concourse import bass_utils

F32 = mybir.dt.float32
BF16 = mybir.dt.bfloat16
AF = mybir.ActivationFunctionType
ALU = mybir.AluOpType

B, T, D, H, DK = 4, 2048, 1024, 16, 64
NC_ = 8          # cores
HPG = 8          # heads per group
NPAIR = 4        # head pairs per core
KT = 8           # 128-row k-tiles over D
XC = 512         # x/qkv t-chunk width
NXC = T // XC    # 4
QC = 512         # attention q-chunk width
NQC = T // QC    # 4
NKB = T // 128   # 16 key blocks

GPS_COMBINE = False  # GpSimd cannot read PSUM (BIR verifier) -> DVE only

_cache = {}


def _build_nc(trace_scopes=False):
    nc = bacc.Bacc("TRN2", target_bir_lowering=False, debug=False)

    xT_d = nc.dram_tensor("xT", [D, T], BF16, kind="ExternalInput").ap()
    wqk_d = nc.dram_tensor("wqk", [D, 1024], BF16, kind="ExternalInput").ap()
    wva_d = nc.dram_tensor("wva", [D, 520], BF16, kind="ExternalInput").ap()
    bva_d = nc.dram_tensor("bva", [1, 520], BF16, kind="ExternalInput").ap()
    ones_d = nc.dram_tensor("ones1", [1, 128], BF16, kind="ExternalInput").ap()
    wo_d = nc.dram_tensor("wo", [512, 1024], BF16, kind="ExternalInput").ap()
    bqk_d = nc.dram_tensor("bqk", [128, 8], F32, kind="ExternalInput").ap()
    cos_d = nc.dram_tensor("cos4", [128, T], BF16, kind="ExternalInput").ap()
    sin_d = nc.dram_tensor("sin4", [128, T], BF16, kind="ExternalInput").ap()
    out_d = nc.dram_tensor("out", [T, 1024], F32, kind="ExternalOutput").ap()

    with tile.TileContext(nc, pool_alloc_mode="queue") as tc:
        _emit(tc, nc, xT_d, wqk_d, wva_d, bva_d, ones_d, wo_d, bqk_d,
              cos_d, sin_d, out_d)
    nc.compile()
    return nc


def _emit(tc, nc, xT_d, wqk_d, wva_d, bva_d, ones_d, wo_d, bqk_d,
          cos_d, sin_d, out_d):
    from contextlib import ExitStack
    ctx = ExitStack()
    with ctx:
        consts = ctx.enter_context(tc.tile_pool(name="consts", bufs=1))
        vpool = ctx.enter_context(tc.tile_pool(name="vpool", bufs=1))
        qkp = ctx.enter_context(tc.tile_pool(name="qkp", bufs=8))
        ep = ctx.enter_context(tc.tile_pool(name="ep", bufs=6))
        zbp = ctx.enter_context(tc.tile_pool(name="zbp", bufs=2))
        atp = ctx.enter_context(tc.tile_pool(name="atp", bufs=16))
        wqkp = ctx.enter_context(tc.tile_pool(name="wqkp", bufs=2))
        xp = ctx.enter_context(tc.tile_pool(name="xp", bufs=2))
        t1p = ctx.enter_context(tc.tile_pool(name="t1p", bufs=4))
        wop = ctx.enter_context(tc.tile_pool(name="wop", bufs=1))
        outp = ctx.enter_context(tc.tile_pool(name="outp", bufs=3))
        # PSUM budget (8 banks): sAB ring2 = 4, mm ring2 = 2, pv ring2 = 2
        ps_mm = ctx.enter_context(tc.tile_pool(name="ps_mm", bufs=2, space="PSUM"))
        ps_s = ctx.enter_context(tc.tile_pool(name="ps_s", bufs=2, space="PSUM"))
        ps_pv = ctx.enter_context(tc.tile_pool(name="ps_pv", bufs=2, space="PSUM"))

        # ---------------- constants ----------------
        cos_t = consts.tile([128, T], BF16, tag="cos")
        sin_t = consts.tile([128, T], BF16, tag="sin")
        bqk_t = consts.tile([128, 8], F32, tag="bqk")
        bva_t = consts.tile([1, 520], BF16, tag="bva")
        ones_t = consts.tile([1, 128], BF16, tag="ones")
        wva_t = consts.tile([128, KT, 520], BF16, tag="wva")
        wo_t = wop.tile([128, 4, 1024], BF16, tag="wo")

        # V_aug tiles per pair-couple: [128 tok, 16 kb, 4 heads * 65]
        V2_ts = [vpool.tile([128, NKB, 260], BF16, tag=f"V{g}", name=f"V{g}")
                 for g in range(2)]

        xT_r = xT_d.rearrange("(k p) t -> p k t", p=128)
        wqk_r = wqk_d.rearrange("(k p) m -> p k m", p=128)

        # live per-pair state
        wqk_pair = [None] * NPAIR
        qp_ts = [None] * NPAIR
        kp_ts = [None] * NPAIR
        at_tiles = [[None] * NQC for _ in range(NPAIR)]
        xc_tiles = {}            # (p, tq) -> xc tile (ring of 2)
        chunk_seq = [(p, tq) for p in range(NPAIR) for tq in range(NXC)]

        def _load_xc(ci):
            p, tq = chunk_seq[ci]
            xc = xp.tile([128, KT, XC], BF16, tag="xc", name=f"xc{p}_{tq}")
            nc.sync.dma_start(out=xc[:], in_=xT_r[:, :, tq * XC:(tq + 1) * XC])
            xc_tiles[(p, tq)] = xc

        def _alloc_pair(p):
            wqk_pair[p] = wqkp.tile([128, KT, 256], BF16, tag="wqk",
                                    name=f"wqk{p}")
            nc.sync.dma_start(out=wqk_pair[p][:],
                              in_=wqk_r[:, :, 256 * p:256 * (p + 1)])
            qp_ts[p] = [qkp.tile([128, QC], BF16, tag=f"qp{p % 2}",
                                 name=f"qp{p}_{i}") for i in range(NQC)]
            kp_ts[p] = [qkp.tile([128, QC], BF16, tag=f"kp{p % 2}",
                                 name=f"kp{p}_{i}") for i in range(NQC)]

        # ---------- QKV projection units (per pair, emission closures) ----
        def u_qk_mms(p, tq, mloc, box):
            def f():
                mmp = ps_mm.tile([128, 512], F32, tag="mm", name="mmqk")
                box[0] = mmp
                xc = xc_tiles[(p, tq)]
                for k in range(KT):
                    nc.tensor.matmul(
                        mmp[:], lhsT=wqk_pair[p][:, k, mloc * 128:(mloc + 1) * 128],
                        rhs=xc[:, k, :], start=(k == 0), stop=(k == KT - 1))
            return f

        def u_rope(p, tq, mloc, box):
            def f():
                mmp = box[0]
                c0 = tq * XC
                msel = 2 * p + mloc
                bcol = bqk_t[:, msel:msel + 1]
                dest = qp_ts[p] if mloc == 0 else kp_ts[p]
                dsl = dest[tq]
                # t1 = (psum + b) * cos  (bf16, SBUF)
                t1 = t1p.tile([128, XC], BF16, tag="t1")
                nc.vector.scalar_tensor_tensor(
                    t1[:], mmp[:], bcol, cos_t[:, c0:c0 + XC],
                    op0=ALU.add, op1=ALU.mult)
                # t2 = (psum + b) * sin, IN-PLACE into the qk psum tile
                # (same-engine FIFO orders it after the t1 read; PSUM
                # operands are exempt from the SBUF same-start rule)
                nc.vector.scalar_tensor_tensor(
                    mmp[:], mmp[:], bcol, sin_t[:, c0:c0 + XC],
                    op0=ALU.add, op1=ALU.mult)
                for hh in range(2):
                    b0 = 64 * hh
                    eng = nc.gpsimd if (GPS_COMBINE and hh == 1) else nc.vector
                    # lo = e*c - o*s ; hi = e*s + o*c
                    eng.tensor_sub(dsl[b0:b0 + 32, :],
                                   t1[b0:b0 + 32, :], mmp[b0 + 32:b0 + 64, :])
                    eng.tensor_add(dsl[b0 + 32:b0 + 64, :],
                                   mmp[b0:b0 + 32, :], t1[b0 + 32:b0 + 64, :])
            return f

        def u_v_tb(p, tq, tb2):
            # computes V_aug for the pair-couple (p, p+1); called for even p
            def f():
                tb = tq * (XC // 128) + tb2
                xc = xc_tiles[(p, tq)]
                pvm = ps_mm.tile([128, 512], F32, tag="mm", name="mmv")
                h0 = 260 * (p // 2)
                for k in range(KT):
                    nc.tensor.matmul(
                        pvm[:, 0:260], lhsT=xc[:, k, tb2 * 128:(tb2 + 1) * 128],
                        rhs=wva_t[:, k, h0:h0 + 260],
                        start=(k == 0), stop=False)
                nc.tensor.matmul(pvm[:, 0:260], lhsT=ones_t[:],
                                 rhs=bva_t[:, h0:h0 + 260],
                                 start=False, stop=True)
                nc.scalar.copy(V2_ts[p // 2][:, tb, :], pvm[:, 0:260])
            return f

        def qkv_chunk_units(p, tq):
            ci = chunk_seq.index((p, tq))
            us = []
            if tq == 0:
                us.append(lambda p=p: _alloc_pair(p))
            box0, box1 = [None], [None]
            us.append(u_qk_mms(p, tq, 0, box0))
            us.append(u_qk_mms(p, tq, 1, box1))
            us.append(u_rope(p, tq, 0, box0))
            us.append(u_rope(p, tq, 1, box1))
            if p % 2 == 0:
                for tb2 in range(XC // 128):
                    us.append(u_v_tb(p, tq, tb2))
            if ci + 2 < len(chunk_seq):
                us.append(lambda ci=ci: _load_xc(ci + 2))
            return us

        # ---------- attention blocks (per pair, emission closures) --------
        def att_blocks(p, qc):
            st = {}
            blocks = [_mk_qc_start(p, qc, st)]
            nkb = 4 * qc + 4
            for kb in range(nkb):
                blocks.append(_mk_block(p, qc, kb, st))
            blocks.append(_mk_qc_end(p, qc, st))
            return blocks

        def _emit_s(p, qc, kb, st):
            d = kb - 4 * qc
            v0 = 0 if d < 0 else min(128 * d, QC - 128)
            sAB = ps_s.tile([128, 2, QC], F32, tag="s")
            kq = kp_ts[p][kb // 4]
            kc0 = (kb % 4) * 128
            qq = qp_ts[p][qc]
            nc.tensor.matmul(sAB[:, 0, v0:], lhsT=kq[0:64, kc0:kc0 + 128],
                             rhs=qq[0:64, v0:],
                             start=True, stop=True, tile_position=(0, 0))
            nc.tensor.matmul(sAB[:, 1, v0:], lhsT=kq[64:128, kc0:kc0 + 128],
                             rhs=qq[64:128, v0:],
                             start=True, stop=True, tile_position=(64, 0))
            st[kb] = (sAB, d, v0)

        def _mk_qc_start(p, qc, st):
            def f():
                st["pvA"] = ps_pv.tile([65, QC], F32, tag="pv", name="pvA")
                st["pvB"] = ps_pv.tile([65, QC], F32, tag="pv", name="pvB")
                _emit_s(p, qc, 0, st)
            return f

        def _mk_block(p, qc, kb, st):
            nkb = 4 * qc + 4

            def f():
                if kb + 1 < nkb:
                    _emit_s(p, qc, kb + 1, st)
                sAB, d, v0 = st.pop(kb)
                # one ACT instr: exp of both heads -> bf16
                e = ep.tile([128, 2, QC], BF16, tag="e")
                nc.scalar.activation(e[:, :, v0:], sAB[:, :, v0:], AF.Exp,
                                     scale=0.125)
                if d >= 0:
                    # zero the not-yet-causal triangle of E on the Pool
                    # engine (scores are bounded so unmasked exp is finite)
                    nc.gpsimd.affine_select(
                        out=e[:, :, v0:v0 + 128], in_=e[:, :, v0:v0 + 128],
                        compare_op=ALU.is_ge, fill=0.0,
                        base=0, pattern=[[0, 2], [1, 128]],
                        channel_multiplier=-1)
                for hh, pv in ((0, st["pvA"]), (1, st["pvB"])):
                    c0 = 130 * (p % 2) + 65 * hh
                    nc.tensor.matmul(pv[0:65, v0:],
                                     lhsT=V2_ts[p // 2][:, kb, c0:c0 + 65],
                                     rhs=e[:, hh, v0:],
                                     start=(kb == 0), stop=(kb == nkb - 1))
            return f

        def _mk_qc_end(p, qc, st):
            def f():
                at = atp.tile([128, QC], BF16, tag="attnT", name=f"at{p}_{qc}")
                at_tiles[p][qc] = at
                pvA, pvB = st["pvA"], st["pvB"]
                zrA = zbp.tile([1, QC], F32, tag="zr", name="zrA")
                zrB = zbp.tile([1, QC], F32, tag="zr", name="zrB")
                nc.vector.tensor_copy(zrA[:], pvA[64:65, :])
                nc.vector.tensor_copy(zrB[:], pvB[64:65, :])
                rzA = zbp.tile([1, QC], F32, tag="rz", name="rzA")
                rzB = zbp.tile([1, QC], F32, tag="rz", name="rzB")
                nc.vector.reciprocal_approx_fast(rzA[:], zrA[:])
                nc.vector.reciprocal_approx_fast(rzB[:], zrB[:])
                zbA = zbp.tile([128, QC], F32, tag="zbA", name="zbA")
                zbB = zbp.tile([128, QC], F32, tag="zbB", name="zbB")
                nc.gpsimd.partition_broadcast(zbA[:], rzA[:])
                nc.gpsimd.partition_broadcast(zbB[:], rzB[:])
                nc.vector.tensor_mul(at[0:64, :], pvA[0:64, :], zbA[0:64, :])
                nc.vector.tensor_mul(at[64:128, :], pvB[0:64, :], zbB[64:128, :])
                if p == NPAIR - 1:
                    for qb in range(4 * qc, 4 * qc + 4):
                        for oc in range(2):
                            fills.append((p, 99, u_out(qb, oc), False))
            return f

        # ---------- output projection units ----------
        def u_out(qb, oc):
            def f():
                po = ps_mm.tile([128, 512], F32, tag="mm", name="mmo")
                for p4 in range(NPAIR):
                    nc.tensor.matmul(
                        po[:],
                        lhsT=at_tiles[p4][qb // 4][:, (qb % 4) * 128:(qb % 4) * 128 + 128],
                        rhs=wo_t[:, p4, oc * 512:(oc + 1) * 512],
                        start=(p4 == 0), stop=(p4 == NPAIR - 1))
                ot = outp.tile([128, 512], F32, tag="ot")
                nc.vector.tensor_copy(ot[:], po[:])
                nc.sync.dma_start(out=out_d[qb * 128:(qb + 1) * 128,
                                            oc * 512:(oc + 1) * 512], in_=ot[:])
            return f

        # ---------- unified gated pipeline emission ----------
        fills = deque()
        gates = [0] * (NPAIR + 1)   # gates[p] = chunks of pair p emitted

        def pump():
            p_, tq_, u, last = fills.popleft()
            u()
            if last:
                gates[p_] = tq_ + 1

        def flush(p, n):
            while gates[p] < n and fills:
                pump()

        # constants on the scalar-engine HWDGE queue, first x/wqk on sync:
        # both queues run in parallel so the first qk matmul starts sooner.
        nc.scalar.dma_start(out=bqk_t[:], in_=bqk_d)
        nc.scalar.dma_start(out=cos_t[:], in_=cos_d)
        nc.scalar.dma_start(out=sin_t[:], in_=sin_d)
        _alloc_pair(0)
        _load_xc(0)
        _load_xc(1)
        nc.scalar.dma_start(out=wva_t[:],
                            in_=wva_d.rearrange("(k p) m -> p k m", p=128))
        nc.scalar.dma_start(out=bva_t[:], in_=bva_d)
        nc.scalar.dma_start(out=ones_t[:], in_=ones_d)

        for p in range(NPAIR):
            for tq in range(NXC):
                us = qkv_chunk_units(p, tq)
                if p == 0 and tq == 0:
                    us = us[1:]   # _alloc_pair(0) already emitted
                fills.extend((p, tq, u, i == len(us) - 1)
                             for i, u in enumerate(us))

        flush(0, 1)
        # out-proj weights needed only from pair-3 attention onwards
        nc.scalar.dma_start(out=wo_t[:],
                            in_=wo_d.rearrange("(k p) m -> p k m", p=128))

        for p in range(NPAIR):
            for qc in range(NQC):
                flush(p, qc + 1)          # hard requirement
                if qc == NQC - 1 and p + 1 < NPAIR:
                    flush(p + 1, 1)       # next pair's first chunk early
                for blk in att_blocks(p, qc):
                    blk()
                    if fills:
                        pump()
                    # catch-up: stay a chunk ahead of the attention sweep
                    if fills and gates[p] < min(qc + 2, NXC):
                        pump()
        while fills:
            pump()


def _prep_inputs(x, W_qkv, b_qkv, W_out, cos, sin):
    """Host-side sharding/permutation. Returns list of 8 per-core in_maps."""
    BF = ml_dtypes.bfloat16
    x = np.ascontiguousarray(np.asarray(x, dtype=np.float32))
    W_qkv = np.asarray(W_qkv, dtype=np.float32)
    b_qkv = np.asarray(b_qkv, dtype=np.float32)
    W_out = np.asarray(W_out, dtype=np.float32)
    cos = np.asarray(cos, dtype=np.float32)
    sin = np.asarray(sin, dtype=np.float32)

    xTs = [np.ascontiguousarray(x[b].T.astype(BF)) for b in range(B)]
    # rope tables: rows r = table[:, r % 32]
    cosT = np.ascontiguousarray(cos.T)           # [32, T]
    sinT = np.ascontiguousarray(sin.T)
    cos4 = np.ascontiguousarray(np.tile(cosT, (4, 1)).astype(BF))   # [128, T]
    sin4 = np.ascontiguousarray(np.tile(sinT, (4, 1)).astype(BF))
    ones1 = np.ones((1, 128), BF)

    groups = []
    for g in range(2):
        heads = [g * HPG + i for i in range(HPG)]
        qk_cols = []
        for p in range(NPAIR):
            A, Bh = heads[2 * p], heads[2 * p + 1]
            for base in (0, DK):                  # q block then k block
                for h in (A, Bh):
                    qk_cols += list(3 * DK * h + base + np.arange(0, DK, 2))
                    qk_cols += list(3 * DK * h + base + np.arange(1, DK, 2))
        qk_cols = np.array(qk_cols)
        wqk = np.ascontiguousarray(W_qkv[:, qk_cols].astype(BF))      # [1024, 1024]
        bqk = np.ascontiguousarray(b_qkv[qk_cols].reshape(8, 128).T)  # [128, 8]
        # v with interleaved ones cols, pair-major: [1024, 8*65]
        wva = np.zeros((D, 520), np.float32)
        bva = np.zeros((1, 520), np.float32)
        for i, h in enumerate(heads):
            vcols = 3 * DK * h + 2 * DK + np.arange(DK)
            wva[:, i * 65:i * 65 + 64] = W_qkv[:, vcols]
            bva[0, i * 65:i * 65 + 64] = b_qkv[vcols]
            bva[0, i * 65 + 64] = 1.0                 # ones column
        wo = np.ascontiguousarray(W_out[g * 512:(g + 1) * 512, :].astype(BF))
        groups.append(dict(wqk=wqk, bqk=bqk,
                           wva=np.ascontiguousarray(wva.astype(BF)),
                           bva=np.ascontiguousarray(bva.astype(BF)), wo=wo))

    in_maps = []
    for c in range(NC_):
        b, g = c // 2, c % 2
        gr = groups[g]
        in_maps.append({
            "xT": xTs[b], "wqk": gr["wqk"], "wva": gr["wva"], "bva": gr["bva"],
            "ones1": ones1, "wo": gr["wo"], "bqk": gr["bqk"],
            "cos4": cos4, "sin4": sin4,
        })
    return in_maps


def run(x, W_qkv, b_qkv, W_out, b_out, cos, sin, trace=False, trace_cores=None):
    """Build/compile (cached), run on 8 cores, return (out, BassKernelResults)."""
    if "nc" not in _cache:
        _cache["nc"] = _build_nc()
    nc = _cache["nc"]
    in_maps = _prep_inputs(x, W_qkv, b_qkv, W_out, cos, sin)
    kw = {}
    if trace:
        kw = dict(trace=True, trace_cores=trace_cores or [0])
    res = bass_utils.run_bass_kernel_spmd(nc, in_maps, core_ids=list(range(NC_)), **kw)
    b_out = np.asarray(b_out, dtype=np.float32)
    out = np.empty((B, T, D), np.float32)
    for b in range(B):
        out[b] = res.results[2 * b]["out"] + res.results[2 * b + 1]["out"] + b_out[None, :]
    return out, res


def kernel(x, W_qkv, b_qkv, W_out, b_out, cos, sin):
    out, _ = run(x, W_qkv, b_qkv, W_out, b_out, cos, sin)
    return out


# revision 5
# speedup vs baseline: 1.0272x; 1.0272x over previous
"""Causal self-attention (B=4, T=2048, D=1024, H=16) on 8 TRN2 NeuronCores.

Sharding: core c handles batch b=c//2 and head-group g=c%2 (8 heads).
Each core computes its heads' attention + a partial output projection
(contraction over its 512 attn channels); the host sums the two partials
per batch and adds b_out.

v3 (from v2 @362us): trace-driven fixes
 - sAB psum ring is now dedicated (rope t2 no longer steals its slots):
   t2 is computed IN-PLACE into the qk-proj psum tile, so S(kb+1) truly
   double-buffers against exp(kb).
 - x chunks are DMA-prefetched one chunk ahead; initial const DMAs go on
   the scalar-engine HWDGE queue in parallel with wqk/x on sync.
 - rope tables are bf16 (halves their DMA); head-B rope combines run on
   GpSimd to cut the DVE critical path.
 - norm chain slimmed: reciprocal reads the Z row straight from PSUM,
   one zb tile with two half broadcasts, no intermediate copies.
 - pair-lookahead gating: next pair's first chunks are pumped during the
   current pair's last q-chunk, so S never waits on rope at boundaries.

Per-core device pipeline (per head-pair p, heads packed 2/128-partitions):
  qk-proj   qkT[ch,T] = wqk.T @ xT  (bf16, ch-major), rope via DVE/Pool
  v-proj    V[t, 2*65] = xT.T @ wv_aug  (65th col per head = ones -> Z)
  S^T       [k,q] = k'^T q' per head, 2 heads in PE quadrants (K=64)
  softmax   exp((S)*0.125) in ONE ACT instr per block (both heads),
            bf16 out; normalizer Z from the V ones col
  PV        attn_aug^T[65,q] = V_aug^T @ E^T accumulated over k blocks
  norm      rz = 1/Z (DVE recip from psum), partition-bcast (Pool),
            attnT = pv * rz -> bf16
  out-proj  out[q,o] = attnT.T @ wo  (partial; host adds pair partials)
"""
import sys
from collections import deque
import numpy as np

for _p in ("/opt/trn_rl_repo", "/root/.axon_site/_ro/trn_rl_repo"):
    if _p not in sys.path:
        sys.path.append(_p)

import ml_dtypes
import concourse.bass as bass
import concourse.bacc as bacc
import concourse.tile as tile
import concourse.mybir as mybir
from concourse import bass_utils

F32 = mybir.dt.float32
BF16 = mybir.dt.bfloat16
AF = mybir.ActivationFunctionType
ALU = mybir.AluOpType

B, T, D, H, DK = 4, 2048, 1024, 16, 64
NC_ = 8          # cores
HPG = 8          # heads per group
NPAIR = 4        # head pairs per core
KT = 8           # 128-row k-tiles over D
XC = 512         # x/qkv t-chunk width
NXC = T // XC    # 4
QC = 512         # attention q-chunk width
NQC = T // QC    # 4
NKB = T // 128   # 16 key blocks

GPS_COMBINE = False  # GpSimd cannot read PSUM (BIR verifier) -> DVE only

_cache = {}


def _build_nc(trace_scopes=False):
    nc = bacc.Bacc("TRN2", target_bir_lowering=False, debug=False)

    xT_d = nc.dram_tensor("xT", [D, T], BF16, kind="ExternalInput").ap()
    wqk_d = nc.dram_tensor("wqk", [D, 1024], BF16, kind="ExternalInput").ap()
    wva_d = nc.dram_tensor("wva", [D, 520], BF16, kind="ExternalInput").ap()
    bva_d = nc.dram_tensor("bva", [1, 520], BF16, kind="ExternalInput").ap()
    ones_d = nc.dram_tensor("ones1", [1, 128], BF16, kind="ExternalInput").ap()
    wo_d = nc.dram_tensor("wo", [512, 1024], BF16, kind="ExternalInput").ap()
    bqk_d = nc.dram_tensor("bqk", [128, 8], F32, kind="ExternalInput").ap()
    cos_d = nc.dram_tensor("cos4", [128, T], BF16, kind="ExternalInput").ap()
    sin_d = nc.dram_tensor("sin4", [128, T], BF16, kind="ExternalInput").ap()
    out_d = nc.dram_tensor("out", [T, 1024], F32, kind="ExternalOutput").ap()

    with tile.TileContext(nc, pool_alloc_mode="queue") as tc:
        _emit(tc, nc, xT_d, wqk_d, wva_d, bva_d, ones_d, wo_d, bqk_d,
              cos_d, sin_d, out_d)
    nc.compile()
    return nc


def _emit(tc, nc, xT_d, wqk_d, wva_d, bva_d, ones_d, wo_d, bqk_d,
          cos_d, sin_d, out_d):
    from contextlib import ExitStack
    ctx = ExitStack()
    with ctx:
        consts = ctx.enter_context(tc.tile_pool(name="consts", bufs=1))
        vpool = ctx.enter_context(tc.tile_pool(name="vpool", bufs=1))
        qkp = ctx.enter_context(tc.tile_pool(name="qkp", bufs=8))
        ep = ctx.enter_context(tc.tile_pool(name="ep", bufs=6))
        zbp = ctx.enter_context(tc.tile_pool(name="zbp", bufs=2))
        atp = ctx.enter_context(tc.tile_pool(name="atp", bufs=16))
        wqkp = ctx.enter_context(tc.tile_pool(name="wqkp", bufs=2))
        xp = ctx.enter_context(tc.tile_pool(name="xp", bufs=2))
        t1p = ctx.enter_context(tc.tile_pool(name="t1p", bufs=4))
        wop = ctx.enter_context(tc.tile_pool(name="wop", bufs=1))
        outp = ctx.enter_context(tc.tile_pool(name="outp", bufs=3))
        # PSUM budget (8 banks): sAB ring2 = 4, mm ring2 = 2, pv ring2 = 2
        ps_mm = ctx.enter_context(tc.tile_pool(name="ps_mm", bufs=2, space="PSUM"))
        ps_s = ctx.enter_context(tc.tile_pool(name="ps_s", bufs=2, space="PSUM"))
        ps_pv = ctx.enter_context(tc.tile_pool(name="ps_pv", bufs=2, space="PSUM"))

        # ---------------- constants ----------------
        cos_t = consts.tile([128, T], BF16, tag="cos")
        sin_t = consts.tile([128, T], BF16, tag="sin")
        bqk_t = consts.tile([128, 8], F32, tag="bqk")
        bva_t = consts.tile([1, 520], BF16, tag="bva")
        ones_t = consts.tile([1, 128], BF16, tag="ones")
        wva_t = consts.tile([128, KT, 520], BF16, tag="wva")
        wo_t = wop.tile([128, 4, 1024], BF16, tag="wo")

        # V_aug tiles per pair-couple: [128 tok, 16 kb, 4 heads * 65]
        V2_ts = [vpool.tile([128, NKB, 260], BF16, tag=f"V{g}", name=f"V{g}")
                 for g in range(2)]

        xT_r = xT_d.rearrange("(k p) t -> p k t", p=128)
        wqk_r = wqk_d.rearrange("(k p) m -> p k m", p=128)

        # live per-pair state
        wqk_pair = [None] * NPAIR
        qp_ts = [None] * NPAIR
        kp_ts = [None] * NPAIR
        at_tiles = [[None] * NQC for _ in range(NPAIR)]
        xc_tiles = {}            # (p, tq) -> xc tile (ring of 2)
        chunk_seq = [(p, tq) for p in range(NPAIR) for tq in range(NXC)]

        def _load_xc(ci):
            p, tq = chunk_seq[ci]
            xc = xp.tile([128, KT, XC], BF16, tag="xc", name=f"xc{p}_{tq}")
            nc.sync.dma_start(out=xc[:], in_=xT_r[:, :, tq * XC:(tq + 1) * XC])
            xc_tiles[(p, tq)] = xc

        def _alloc_pair(p):
            wqk_pair[p] = wqkp.tile([128, KT, 256], BF16, tag="wqk",
                                    name=f"wqk{p}")
            nc.sync.dma_start(out=wqk_pair[p][:],
                              in_=wqk_r[:, :, 256 * p:256 * (p + 1)])
            qp_ts[p] = [qkp.tile([128, QC], BF16, tag=f"qp{p % 2}",
                                 name=f"qp{p}_{i}") for i in range(NQC)]
            kp_ts[p] = [qkp.tile([128, QC], BF16, tag=f"kp{p % 2}",
                                 name=f"kp{p}_{i}") for i in range(NQC)]

        # ---------- QKV projection units (per pair, emission closures) ----
        def u_qk_mms(p, tq, mloc, box):
            def f():
                mmp = ps_mm.tile([128, 512], F32, tag="mm", name="mmqk")
                box[0] = mmp
                xc = xc_tiles[(p, tq)]
                for k in range(KT):
                    nc.tensor.matmul(
                        mmp[:], lhsT=wqk_pair[p][:, k, mloc * 128:(mloc + 1) * 128],
                        rhs=xc[:, k, :], start=(k == 0), stop=(k == KT - 1))
            return f

        def u_rope(p, tq, mloc, box):
            def f():
                mmp = box[0]
                c0 = tq * XC
                msel = 2 * p + mloc
                bcol = bqk_t[:, msel:msel + 1]
                dest = qp_ts[p] if mloc == 0 else kp_ts[p]
                dsl = dest[tq]
                # t1 = (psum + b) * cos  (bf16, SBUF)
                t1 = t1p.tile([128, XC], BF16, tag="t1")
                nc.vector.scalar_tensor_tensor(
                    t1[:], mmp[:], bcol, cos_t[:, c0:c0 + XC],
                    op0=ALU.add, op1=ALU.mult)
                # t2 lives in PSUM (s-ring): cross-partition reads are exempt
                # from the SBUF same-start-partition rule; keeping it out of
                # the mm ring keeps fill matmuls off the rope critical path.
                t2 = ps_s.tile([128, XC], F32, tag="s", name="t2")
                nc.vector.scalar_tensor_tensor(
                    t2[:], mmp[:], bcol, sin_t[:, c0:c0 + XC],
                    op0=ALU.add, op1=ALU.mult)
                for hh in range(2):
                    b0 = 64 * hh
                    # lo = e*c - o*s ; hi = e*s + o*c
                    nc.vector.tensor_sub(dsl[b0:b0 + 32, :],
                                         t1[b0:b0 + 32, :], t2[b0 + 32:b0 + 64, :])
                    nc.vector.tensor_add(dsl[b0 + 32:b0 + 64, :],
                                         t2[b0:b0 + 32, :], t1[b0 + 32:b0 + 64, :])
            return f

        def u_v_tb(p, tq, tb2):
            # computes V_aug for the pair-couple (p, p+1); called for even p
            def f():
                tb = tq * (XC // 128) + tb2
                xc = xc_tiles[(p, tq)]
                pvm = ps_mm.tile([128, 512], F32, tag="mm", name="mmv")
                h0 = 260 * (p // 2)
                for k in range(KT):
                    nc.tensor.matmul(
                        pvm[:, 0:260], lhsT=xc[:, k, tb2 * 128:(tb2 + 1) * 128],
                        rhs=wva_t[:, k, h0:h0 + 260],
                        start=(k == 0), stop=False)
                nc.tensor.matmul(pvm[:, 0:260], lhsT=ones_t[:],
                                 rhs=bva_t[:, h0:h0 + 260],
                                 start=False, stop=True)
                nc.scalar.copy(V2_ts[p // 2][:, tb, :], pvm[:, 0:260])
            return f

        def qkv_chunk_units(p, tq):
            ci = chunk_seq.index((p, tq))
            us = []
            if tq == 0:
                us.append(lambda p=p: _alloc_pair(p))
            box0, box1 = [None], [None]
            us.append(u_qk_mms(p, tq, 0, box0))
            us.append(u_qk_mms(p, tq, 1, box1))
            us.append(u_rope(p, tq, 0, box0))
            us.append(u_rope(p, tq, 1, box1))
            if p % 2 == 0:
                for tb2 in range(XC // 128):
                    us.append(u_v_tb(p, tq, tb2))
            if ci + 2 < len(chunk_seq):
                us.append(lambda ci=ci: _load_xc(ci + 2))
            return us

        # ---------- attention blocks (per pair, emission closures) --------
        def att_blocks(p, qc):
            st = {}
            blocks = [_mk_qc_start(p, qc, st)]
            nkb = 4 * qc + 4
            for kb in range(nkb):
                blocks.append(_mk_block(p, qc, kb, st))
            blocks.append(_mk_qc_end(p, qc, st))
            return blocks

        def _emit_s(p, qc, kb, st):
            d = kb - 4 * qc
            v0 = 0 if d < 0 else min(128 * d, QC - 128)
            sAB = ps_s.tile([128, 2, QC], F32, tag="s")
            kq = kp_ts[p][kb // 4]
            kc0 = (kb % 4) * 128
            qq = qp_ts[p][qc]
            nc.tensor.matmul(sAB[:, 0, v0:], lhsT=kq[0:64, kc0:kc0 + 128],
                             rhs=qq[0:64, v0:],
                             start=True, stop=True, tile_position=(0, 0))
            nc.tensor.matmul(sAB[:, 1, v0:], lhsT=kq[64:128, kc0:kc0 + 128],
                             rhs=qq[64:128, v0:],
                             start=True, stop=True, tile_position=(64, 0))
            st[kb] = (sAB, d, v0)

        def _mk_qc_start(p, qc, st):
            def f():
                st["pvA"] = ps_pv.tile([65, QC], F32, tag="pv", name="pvA")
                st["pvB"] = ps_pv.tile([65, QC], F32, tag="pv", name="pvB")
                _emit_s(p, qc, 0, st)
            return f

        def _mk_block(p, qc, kb, st):
            nkb = 4 * qc + 4

            def f():
                if kb + 1 < nkb:
                    _emit_s(p, qc, kb + 1, st)
                sAB, d, v0 = st.pop(kb)
                # one ACT instr: exp of both heads -> bf16
                e = ep.tile([128, 2, QC], BF16, tag="e")
                nc.scalar.activation(e[:, :, v0:], sAB[:, :, v0:], AF.Exp,
                                     scale=0.125)
                if d >= 0:
                    # zero the not-yet-causal triangle of E on the Pool
                    # engine (scores are bounded so unmasked exp is finite)
                    nc.gpsimd.affine_select(
                        out=e[:, :, v0:v0 + 128], in_=e[:, :, v0:v0 + 128],
                        compare_op=ALU.is_ge, fill=0.0,
                        base=0, pattern=[[0, 2], [1, 128]],
                        channel_multiplier=-1)
                for hh, pv in ((0, st["pvA"]), (1, st["pvB"])):
                    c0 = 130 * (p % 2) + 65 * hh
                    nc.tensor.matmul(pv[0:65, v0:],
                                     lhsT=V2_ts[p // 2][:, kb, c0:c0 + 65],
                                     rhs=e[:, hh, v0:],
                                     start=(kb == 0), stop=(kb == nkb - 1))
            return f

        def _mk_qc_end(p, qc, st):
            def f():
                at = atp.tile([128, QC], BF16, tag="attnT", name=f"at{p}_{qc}")
                at_tiles[p][qc] = at
                pvA, pvB = st["pvA"], st["pvB"]
                zrA = zbp.tile([1, QC], F32, tag="zr", name="zrA")
                zrB = zbp.tile([1, QC], F32, tag="zr", name="zrB")
                nc.vector.tensor_copy(zrA[:], pvA[64:65, :])
                nc.vector.tensor_copy(zrB[:], pvB[64:65, :])
                rzA = zbp.tile([1, QC], F32, tag="rz", name="rzA")
                rzB = zbp.tile([1, QC], F32, tag="rz", name="rzB")
                nc.vector.reciprocal_approx_fast(rzA[:], zrA[:])
                nc.vector.reciprocal_approx_fast(rzB[:], zrB[:])
                zbA = zbp.tile([128, QC], F32, tag="zbA", name="zbA")
                zbB = zbp.tile([128, QC], F32, tag="zbB", name="zbB")
                nc.gpsimd.partition_broadcast(zbA[:], rzA[:])
                nc.gpsimd.partition_broadcast(zbB[:], rzB[:])
                nc.vector.tensor_mul(at[0:64, :], pvA[0:64, :], zbA[0:64, :])
                nc.vector.tensor_mul(at[64:128, :], pvB[0:64, :], zbB[64:128, :])
                if p == NPAIR - 1:
                    for qb in range(4 * qc, 4 * qc + 4):
                        for oc in range(2):
                            fills.append((p, 99, u_out(qb, oc), False))
            return f

        # ---------- output projection units ----------
        def u_out(qb, oc):
            def f():
                po = ps_mm.tile([128, 512], F32, tag="mm", name="mmo")
                for p4 in range(NPAIR):
                    nc.tensor.matmul(
                        po[:],
                        lhsT=at_tiles[p4][qb // 4][:, (qb % 4) * 128:(qb % 4) * 128 + 128],
                        rhs=wo_t[:, p4, oc * 512:(oc + 1) * 512],
                        start=(p4 == 0), stop=(p4 == NPAIR - 1))
                ot = outp.tile([128, 512], F32, tag="ot")
                nc.vector.tensor_copy(ot[:], po[:])
                nc.sync.dma_start(out=out_d[qb * 128:(qb + 1) * 128,
                                            oc * 512:(oc + 1) * 512], in_=ot[:])
            return f

        # ---------- unified gated pipeline emission ----------
        fills = deque()
        gates = [0] * (NPAIR + 1)   # gates[p] = chunks of pair p emitted

        def pump():
            p_, tq_, u, last = fills.popleft()
            u()
            if last:
                gates[p_] = tq_ + 1

        def flush(p, n):
            while gates[p] < n and fills:
                pump()

        # constants on the scalar-engine HWDGE queue, first x/wqk on sync:
        # both queues run in parallel so the first qk matmul starts sooner.
        nc.scalar.dma_start(out=bqk_t[:], in_=bqk_d)
        nc.scalar.dma_start(out=cos_t[:], in_=cos_d)
        nc.scalar.dma_start(out=sin_t[:], in_=sin_d)
        _alloc_pair(0)
        _load_xc(0)
        _load_xc(1)
        nc.scalar.dma_start(out=wva_t[:],
                            in_=wva_d.rearrange("(k p) m -> p k m", p=128))
        nc.scalar.dma_start(out=bva_t[:], in_=bva_d)
        nc.scalar.dma_start(out=ones_t[:], in_=ones_d)

        for p in range(NPAIR):
            for tq in range(NXC):
                us = qkv_chunk_units(p, tq)
                if p == 0 and tq == 0:
                    us = us[1:]   # _alloc_pair(0) already emitted
                fills.extend((p, tq, u, i == len(us) - 1)
                             for i, u in enumerate(us))

        flush(0, 1)
        # out-proj weights needed only from pair-3 attention onwards
        nc.scalar.dma_start(out=wo_t[:],
                            in_=wo_d.rearrange("(k p) m -> p k m", p=128))

        for p in range(NPAIR):
            for qc in range(NQC):
                flush(p, qc + 1)          # hard requirement
                if qc == NQC - 1 and p + 1 < NPAIR:
                    flush(p + 1, 1)       # next pair's first chunk early
                for blk in att_blocks(p, qc):
                    blk()
                    if fills:
                        pump()
                    # catch-up: stay a chunk ahead of the attention sweep
                    if fills and gates[p] < min(qc + 2, NXC):
                        pump()
        while fills:
            pump()


def _prep_inputs(x, W_qkv, b_qkv, W_out, cos, sin):
    """Host-side sharding/permutation. Returns list of 8 per-core in_maps."""
    BF = ml_dtypes.bfloat16
    x = np.ascontiguousarray(np.asarray(x, dtype=np.float32))
    W_qkv = np.asarray(W_qkv, dtype=np.float32)
    b_qkv = np.asarray(b_qkv, dtype=np.float32)
    W_out = np.asarray(W_out, dtype=np.float32)
    cos = np.asarray(cos, dtype=np.float32)
    sin = np.asarray(sin, dtype=np.float32)

    xTs = [np.ascontiguousarray(x[b].T.astype(BF)) for b in range(B)]
    # rope tables: rows r = table[:, r % 32]
    cosT = np.ascontiguousarray(cos.T)           # [32, T]
    sinT = np.ascontiguousarray(sin.T)
    cos4 = np.ascontiguousarray(np.tile(cosT, (4, 1)).astype(BF))   # [128, T]
    sin4 = np.ascontiguousarray(np.tile(sinT, (4, 1)).astype(BF))
    ones1 = np.ones((1, 128), BF)

    groups = []
    for g in range(2):
        heads = [g * HPG + i for i in range(HPG)]
        qk_cols = []
        for p in range(NPAIR):
            A, Bh = heads[2 * p], heads[2 * p + 1]
            for base in (0, DK):                  # q block then k block
                for h in (A, Bh):
                    qk_cols += list(3 * DK * h + base + np.arange(0, DK, 2))
                    qk_cols += list(3 * DK * h + base + np.arange(1, DK, 2))
        qk_cols = np.array(qk_cols)
        wqk = np.ascontiguousarray(W_qkv[:, qk_cols].astype(BF))      # [1024, 1024]
        bqk = np.ascontiguousarray(b_qkv[qk_cols].reshape(8, 128).T)  # [128, 8]
        # v with interleaved ones cols, pair-major: [1024, 8*65]
        wva = np.zeros((D, 520), np.float32)
        bva = np.zeros((1, 520), np.float32)
        for i, h in enumerate(heads):
            vcols = 3 * DK * h + 2 * DK + np.arange(DK)
            wva[:, i * 65:i * 65 + 64] = W_qkv[:, vcols]
            bva[0, i * 65:i * 65 + 64] = b_qkv[vcols]
            bva[0, i * 65 + 64] = 1.0                 # ones column
        wo = np.ascontiguousarray(W_out[g * 512:(g + 1) * 512, :].astype(BF))
        groups.append(dict(wqk=wqk, bqk=bqk,
                           wva=np.ascontiguousarray(wva.astype(BF)),
                           bva=np.ascontiguousarray(bva.astype(BF)), wo=wo))

    in_maps = []
    for c in range(NC_):
        b, g = c // 2, c % 2
        gr = groups[g]
        in_maps.append({
            "xT": xTs[b], "wqk": gr["wqk"], "wva": gr["wva"], "bva": gr["bva"],
            "ones1": ones1, "wo": gr["wo"], "bqk": gr["bqk"],
            "cos4": cos4, "sin4": sin4,
        })
    return in_maps


def run(x, W_qkv, b_qkv, W_out, b_out, cos, sin, trace=False, trace_cores=None):
    """Build/compile (cached), run on 8 cores, return (out, BassKernelResults)."""
    if "nc" not in _cache:
        _cache["nc"] = _build_nc()
    nc = _cache["nc"]
    in_maps = _prep_inputs(x, W_qkv, b_qkv, W_out, cos, sin)
    kw = {}
    if trace:
        kw = dict(trace=True, trace_cores=trace_cores or [0])
    res = bass_utils.run_bass_kernel_spmd(nc, in_maps, core_ids=list(range(NC_)), **kw)
    b_out = np.asarray(b_out, dtype=np.float32)
    out = np.empty((B, T, D), np.float32)
    for b in range(B):
        out[b] = res.results[2 * b]["out"] + res.results[2 * b + 1]["out"] + b_out[None, :]
    return out, res


def kernel(x, W_qkv, b_qkv, W_out, b_out, cos, sin):
    out, _ = run(x, W_qkv, b_qkv, W_out, b_out, cos, sin)
    return out
